# revision 1
# baseline (speedup 1.0000x reference)
"""Trainium2 Bass kernel for nn_H_ATT (GatedTrans pair-attention block).

Math (per example):
  HE = tanh(hist@W_hy+b_hy) * lrelu(hist@W_hg+b_hg)      [R, H]
  QE = tanh(ques@W_qy+b_qy) * lrelu(ques@W_qg+b_qg)      [R, H]
  num[q,h]  = sum_k QE[q,k]*W_att[k]*HE[h,k]
  den[q,h]  = sqrt(sum_k QE[q,k]^2 * HE[h,k]^2)
  s = num / max(den, eps)          (b_att cancels in softmax)
  att = causal_softmax(s)          (softmax*tril/renorm == masked softmax)
  feat = att @ hist                 [R, 2H]

Sharding: pure data parallel, 8 examples per core on 8 NeuronCores.
The host pre-transposes activations and pre-blocks weights so every DMA is
contiguous-line friendly; the big GEMMs run as lhsT.T@rhs with K=IN on the
partition dim.
"""

import numpy as np
import ml_dtypes

import bass_rust
import concourse.bass as bass
import concourse.mybir as mybir
import concourse.tile as tile
from concourse.vector_clock import ScopedClock

# ---------------------------------------------------------------------------
# Workaround: this walrus build accepts only ONE semaphore wait on an SP
# Drain, but TileContext's tail drain carries one wait per live semaphore.
# Split them across a chain of drains.
# ---------------------------------------------------------------------------


def _patched_drain_and_barrier(self, tick_clock, wait_clock):
    nc = self.nc
    drain_inst = nc.sync.drain()
    wait_clock.add_sem_waits(
        drain_inst.ins, ScopedClock({None: tick_clock.global_clock})
    )
    waits = list(drain_inst.ins.sync_info.on_wait)
    if len(waits) > 1:
        drain_inst.ins.sync_info = bass_rust.SyncInfo(
            on_wait=waits[:1], on_update=list(drain_inst.ins.sync_info.on_update)
        )
        for i in range(1, len(waits)):
            extra = nc.sync.drain()
            extra.ins.sync_info = bass_rust.SyncInfo(
                on_wait=waits[i : i + 1], on_update=[]
            )
    nc.all_engine_barrier()
    assert self.sems is not None
    popped = nc._tile_sem_poison_stack.pop()
    assert popped is self._sem_poison
    nc.clear_and_free_semaphores(list(self.sems.allocated().values()))
    nc.all_engine_barrier()


tile.TileContext._drain_and_barrier = _patched_drain_and_barrier


def _split_multi_waits(nc):
    """This walrus build accepts at most one semaphore wait per instruction.
    Hoist extra waits onto standalone EventSemaphore instructions inserted
    just before the owning instruction in the same engine's stream."""
    uid = [0]
    for f in nc.m.functions:
        for bb in f.blocks:
            out = []
            for inst in bb.instructions:
                si = inst.sync_info
                if si is not None and len(si.on_wait) > 1:
                    waits = list(si.on_wait)
                    for w in waits[:-1]:
                        nop = mybir.InstEventSemaphore(
                            name=f"I-waitsplit-{uid[0]}", ins=[], outs=[]
                        )
                        uid[0] += 1
                        nop.engine = inst.engine
                        nop.sync_info = bass_rust.SyncInfo(
                            on_wait=[w], on_update=[]
                        )
                        out.append(nop)
                    inst.sync_info = bass_rust.SyncInfo(
                        on_wait=[waits[-1]], on_update=list(si.on_update)
                    )
                out.append(inst)
            bb.instructions[:] = out

# ---------------------------------------------------------------------------

B, R, H, IN = 64, 32, 1024, 2048
NCORES = 8
BL = B // NCORES  # examples per core
BR = BL * R  # 256 rows per core
KC = IN // 128  # 16 contraction chunks
MC = H // 128  # 8 h chunks
NEG = -1.0e30

F32 = mybir.dt.float32


def build_program(mode="f32r", zero_bias=True):
    """Build the per-core Bass program. mode in {"f32r", "bf16"} selects the
    dtype of the big-GEMM operands (weights + transposed activations)."""
    xdt = mybir.dt.float32r if mode == "f32r" else mybir.dt.bfloat16
    FEAT_DT = mybir.dt.float32r

    nc = bass.Bass()
    qt_d = nc.dram_tensor("qt", [KC, 128, BR], xdt, kind="ExternalInput")
    ht_d = nc.dram_tensor("ht", [KC, 128, BR], xdt, kind="ExternalInput")
    hn_d = nc.dram_tensor("hn", [2, 128, IN], FEAT_DT, kind="ExternalInput")
    wh_d = nc.dram_tensor("wh", [MC, 2, 128, KC, 128], xdt, kind="ExternalInput")
    wq_d = nc.dram_tensor("wq", [MC, 2, 128, KC, 128], xdt, kind="ExternalInput")
    b_d = {
        n: nc.dram_tensor(n, [128, MC], F32, kind="ExternalInput")
        for n in ("bhy", "bhg", "bqy", "bqg")
    }
    watt_d = nc.dram_tensor("watt", [128, MC], F32, kind="ExternalInput")
    mask_d = nc.dram_tensor("mask", [128, 128], F32, kind="ExternalInput")
    ident_d = nc.dram_tensor("ident", [128, 128], F32, kind="ExternalInput")
    feat_d = nc.dram_tensor("feat", [2, 128, IN], F32, kind="ExternalOutput")

    ACT = mybir.ActivationFunctionType

    with tile.TileContext(nc) as tc:
        with (
            tc.tile_pool(name="big", bufs=1) as big,
            tc.tile_pool(name="wts", bufs=5) as wts,
            tc.tile_pool(name="tmp", bufs=3) as tmp,
            tc.tile_pool(name="sm", bufs=1) as sm,
        ):
            # ques-transposed activations: needed first
            qt = big.tile([128, KC, BR], xdt, tag="qt")
            for q4 in range(4):
                ks = slice(4 * q4, 4 * (q4 + 1))
                nc.sync.dma_start(
                    qt[:, ks, :], qt_d[ks].rearrange("k p b -> p k b")
                )

            EDT = mybir.dt.bfloat16
            he = big.tile([128, MC, BR], EDT, tag="he")
            he2 = big.tile([128, MC, BR], EDT, tag="he2")
            qew = big.tile([128, MC, BR], EDT, tag="qew")
            qe2 = big.tile([128, MC, BR], EDT, tag="qe2")

            with (
                tc.tile_pool(name="pse", bufs=2, space="PSUM") as pse,
                tc.tile_pool(name="psnd", bufs=1, space="PSUM") as psnd,
            ):
                num_ps = [psnd.tile([128, 128], F32, name=f"num{g}", tag=f"num{g}") for g in range(2)]
                den_ps = [psnd.tile([128, 128], F32, name=f"den{g}", tag=f"den{g}") for g in range(2)]

                def gated(xt, w_dram, by, bg, m):
                    """One fused y+g weight DMA; returns (ty, tg) [128, BR]."""
                    wt = wts.tile([128, 2, KC, 128], xdt, tag="wt")
                    for h2 in range(2):
                        ks = slice(8 * h2, 8 * (h2 + 1))
                        nc.sync.dma_start(
                            wt[:, :, ks, :],
                            w_dram[m, :, :, ks].rearrange("y p k h -> p y k h"),
                        )
                    psy = pse.tile([128, BR], F32, tag="psy")
                    for k in range(KC):
                        nc.tensor.matmul(
                            psy[:], wt[:, 0, k, :], xt[:, k, :],
                            start=(k == 0), stop=(k == KC - 1),
                        )
                    psg = pse.tile([128, BR], F32, tag="psg")
                    for k in range(KC):
                        nc.tensor.matmul(
                            psg[:], wt[:, 1, k, :], xt[:, k, :],
                            start=(k == 0), stop=(k == KC - 1),
                        )
                    ty = tmp.tile([128, BR], F32, tag="ty")
                    nc.scalar.activation(ty[:], psy[:], ACT.Tanh, bias=by[:, m : m + 1])
                    t1 = tmp.tile([128, BR], F32, tag="t1")
                    tg = tmp.tile([128, BR], F32, tag="tg")
                    if zero_bias:
                        # leaky_relu(x) = max(x, 0.01x)
                        nc.vector.tensor_scalar_mul(t1[:], psg[:], 0.01)
                        nc.vector.tensor_max(tg[:], psg[:], t1[:])
                    else:
                        # leaky_relu(x+b) = max(x+b, 0.01*(x+b))
                        nc.vector.tensor_scalar(
                            t1[:], psg[:], bg[:, m : m + 1], 0.01,
                            op0=mybir.AluOpType.add, op1=mybir.AluOpType.mult,
                        )
                        nc.vector.tensor_scalar_add(tg[:], psg[:], bg[:, m : m + 1])
                        nc.vector.tensor_max(tg[:], tg[:], t1[:])
                    return ty, tg

                # consts land while the first ques matmuls run
                bsb = {}
                for n in ("bqy", "bqg", "bhy", "bhg"):
                    bsb[n] = sm.tile([128, MC], F32, name=n, tag=n)
                    nc.sync.dma_start(bsb[n][:], b_d[n][:])
                watt = sm.tile([128, MC], F32, tag="watt")
                nc.sync.dma_start(watt[:], watt_d[:])

                # ques embeddings (first: only needs qt + wq)
                for m in range(MC):
                    ty, tg = gated(qt, wq_d, bsb["bqy"], bsb["bqg"], m)
                    nc.vector.scalar_tensor_tensor(
                        qew[:, m, :], ty[:], watt[:, m : m + 1], tg[:],
                        op0=mybir.AluOpType.mult, op1=mybir.AluOpType.mult,
                    )
                    qe = tmp.tile([128, BR], F32, tag="qe")
                    nc.vector.tensor_mul(qe[:], ty[:], tg[:])
                    nc.scalar.square(qe2[:, m, :], qe[:])
                    if m == 0:
                        # hist-transposed activations: stream in during ques phase
                        ht = big.tile([128, KC, BR], xdt, tag="ht")
                        nc.sync.dma_start(ht[:], ht_d[:].rearrange("k p b -> p k b"))

                # hist embeddings + num/den accumulation per chunk
                for m in range(MC):
                    ty, tg = gated(ht, wh_d, bsb["bhy"], bsb["bhg"], m)
                    nc.vector.tensor_mul(he[:, m, :], ty[:], tg[:])
                    nc.scalar.square(he2[:, m, :], he[:, m, :])
                    for g in range(2):
                        sl = slice(128 * g, 128 * (g + 1))
                        nc.tensor.matmul(
                            num_ps[g][:], qew[:, m, sl], he[:, m, sl],
                            start=(m == 0), stop=(m == MC - 1),
                        )
                        nc.tensor.matmul(
                            den_ps[g][:], qe2[:, m, sl], he2[:, m, sl],
                            start=(m == 0), stop=(m == MC - 1),
                        )
                    if m == 0:
                        # feat inputs: stream in during hist phase
                        hn = big.tile([128, 2, IN], FEAT_DT, tag="hn")
                        nc.sync.dma_start(hn[:], hn_d[:].rearrange("t p d -> p t d"))
                        mask = sm.tile([128, 128], F32, tag="mask")
                        nc.sync.dma_start(mask[:], mask_d[:])
                        ident = sm.tile([128, 128], F32, tag="ident")
                        nc.sync.dma_start(ident[:], ident_d[:])

                # scores while num/den PSUM is still available
                sc = []
                for g in range(2):
                    sd = tmp.tile([128, 128], F32, tag="sd")
                    nc.scalar.activation(sd[:], den_ps[g][:], ACT.Sqrt)
                    rd = tmp.tile([128, 128], F32, tag="rd")
                    nc.vector.reciprocal(rd[:], sd[:])
                    s = sm.tile([128, 128], F32, name=f"sc{g}", tag=f"sc{g}")
                    nc.vector.tensor_mul(s[:], num_ps[g][:], rd[:])
                    nc.vector.tensor_add(s[:], s[:], mask[:])
                    sc.append(s)

            # attention tail + feat
            with (
                tc.tile_pool(name="psa", bufs=1, space="PSUM") as psa,
                tc.tile_pool(name="psf", bufs=4, space="PSUM") as psf,
            ):
                for g in range(2):
                    s = sc[g]
                    att = sm.tile([128, 128], F32, name=f"att{g}", tag=f"att{g}")
                    nc.vector.memset(att[:], 0.0)
                    rs = sm.tile([128, 1], F32, name=f"rs{g}", tag=f"rs{g}")
                    for e in range(4):
                        bl = slice(32 * e, 32 * (e + 1))
                        nc.scalar.activation(att[bl, bl], s[bl, bl], ACT.Exp)
                        nc.vector.reduce_sum(
                            rs[bl, :], att[bl, bl], axis=mybir.AxisListType.X
                        )
                    rrs = sm.tile([128, 1], F32, name=f"rrs{g}", tag=f"rrs{g}")
                    nc.vector.reciprocal(rrs[:], rs[:])
                    nc.vector.tensor_scalar_mul(att[:], att[:], rrs[:])
                    atp = psa.tile([128, 128], F32, tag="atp")
                    nc.tensor.transpose(atp[:], att[:], ident[:])
                    atb = sm.tile([128, 128], FEAT_DT, name=f"atb{g}", tag=f"atb{g}")
                    nc.scalar.copy(atb[:], atp[:])
                    for c2 in range(2):
                        fsb = tmp.tile([128, 1024], F32, tag="fsb")
                        for half in range(2):
                            c = 2 * c2 + half
                            cs = slice(512 * c, 512 * (c + 1))
                            fps = psf.tile([128, 512], F32, tag="fps")
                            nc.tensor.matmul(
                                fps[:], atb[:], hn[:, g, cs], start=True, stop=True
                            )
                            dst = fsb[:, 512 * half : 512 * (half + 1)]
                            if half == 0:
                                nc.scalar.copy(dst, fps[:])
                            else:
                                nc.vector.tensor_copy(dst, fps[:])
                        nc.sync.dma_start(
                            feat_d[g, :, 1024 * c2 : 1024 * (c2 + 1)], fsb[:]
                        )

    _split_multi_waits(nc)
    return nc


# ---------------------------------------------------------------------------
# Host side
# ---------------------------------------------------------------------------

_PROG_CACHE = {}


def _get_prog(mode, zero_bias):
    key = (mode, zero_bias)
    if key not in _PROG_CACHE:
        _PROG_CACHE[key] = build_program(mode, zero_bias)
    return _PROG_CACHE[key]


def _prep_shared(W_hy, b_hy, W_hg, b_hg, W_qy, b_qy, W_qg, b_qg, W_att, mode):
    xnp = np.float32 if mode == "f32r" else ml_dtypes.bfloat16

    def reblock(W):
        # [IN, H] -> [MC, 128, KC, 128]; Wr[m, p, k, h] = W[128k+p, 128m+h]
        return np.ascontiguousarray(
            W.reshape(KC, 128, MC, 128).transpose(2, 1, 0, 3)
        ).astype(xnp)

    def bvec(b):
        return np.ascontiguousarray(b.reshape(MC, 128).T).astype(np.float32)

    m32 = np.where(
        np.arange(32)[None, :] <= np.arange(32)[:, None], 0.0, NEG
    ).astype(np.float32)
    mask = np.tile(m32, (4, 4))
    wh = np.ascontiguousarray(np.stack([reblock(W_hy), reblock(W_hg)], axis=1))
    wq = np.ascontiguousarray(np.stack([reblock(W_qy), reblock(W_qg)], axis=1))
    shared = {
        "wh": wh,
        "wq": wq,
        "bhy": bvec(b_hy),
        "bhg": bvec(b_hg),
        "bqy": bvec(b_qy),
        "bqg": bvec(b_qg),
        "watt": bvec(W_att),
        "mask": np.ascontiguousarray(mask),
        "ident": np.eye(128, dtype=np.float32),
    }
    return shared, xnp


def kernel(
    hist, ques, W_hy, b_hy, W_hg, b_hg, W_qy, b_qy, W_qg, b_qg, W_att, b_att,
    mode="f32r", trace=False,
):
    from concourse.bass_utils import run_bass_kernel_spmd

    hist = np.asarray(hist, np.float32)
    ques = np.asarray(ques, np.float32)
    zero_bias = all(
        not np.any(np.asarray(b)) for b in (b_hy, b_hg, b_qy, b_qg)
    )
    nc = _get_prog(mode, zero_bias)
    shared, xnp = _prep_shared(
        np.asarray(W_hy, np.float32), np.asarray(b_hy, np.float32),
        np.asarray(W_hg, np.float32), np.asarray(b_hg, np.float32),
        np.asarray(W_qy, np.float32), np.asarray(b_qy, np.float32),
        np.asarray(W_qg, np.float32), np.asarray(b_qg, np.float32),
        np.asarray(W_att, np.float32), mode,
    )
    in_maps = []
    for c in range(NCORES):
        hs = hist[c * BL : (c + 1) * BL].reshape(BR, IN)
        qs = ques[c * BL : (c + 1) * BL].reshape(BR, IN)
        im = dict(shared)
        im["qt"] = np.ascontiguousarray(qs.T).reshape(KC, 128, BR).astype(xnp)
        im["ht"] = np.ascontiguousarray(hs.T).reshape(KC, 128, BR).astype(xnp)
        im["hn"] = np.ascontiguousarray(hs.reshape(2, 128, IN))
        in_maps.append(im)

    res = run_bass_kernel_spmd(
        nc, in_maps, core_ids=list(range(NCORES)), trace=trace
    )
    feat = np.concatenate(
        [r["feat"].reshape(BL, R, IN) for r in res.results], axis=0
    )
    if trace:
        return feat, res
    return feat



# revision 2
# speedup vs baseline: 1.4415x; 1.4415x over previous
"""Trainium2 Bass kernel for nn_H_ATT (GatedTrans pair-attention block).

Math (per example):
  HE = tanh(hist@W_hy+b_hy) * lrelu(hist@W_hg+b_hg)      [R, H]
  QE = tanh(ques@W_qy+b_qy) * lrelu(ques@W_qg+b_qg)      [R, H]
  num[q,h]  = sum_k QE[q,k]*W_att[k]*HE[h,k]
  den[q,h]  = sqrt(sum_k QE[q,k]^2 * HE[h,k]^2)
  s = num / max(den, eps)          (b_att cancels in softmax)
  att = causal_softmax(s)          (softmax*tril/renorm == masked softmax)
  feat = att @ hist                 [R, 2H]

Sharding: pure data parallel, 8 examples per core on 8 NeuronCores.

Perf structure (fp8 mode, default):
- All DRAM operand layouts are partition-major so every DMA reads long
  contiguous runs per partition (the f32r baseline moved 512B packets).
- The 4 big embedding GEMMs run as fp8e4 DoubleRow matmuls (2 k-tiles
  per instruction, 2x PE throughput). Weights are pre-scaled by 64 on
  the host so W*64 ~ N(0,1.4) sits in e4m3's normal range; the scale
  is undone exactly: tanh gets scale=1/64, leaky_relu is positively
  homogeneous so the 64x rides through and cancels against watt/4096
  in num and scale=1/64 inside the squares for den.
- hist for the final feat matmul and the output are bf16.
"""

import numpy as np
import ml_dtypes

import bass_rust
import concourse.bass as bass
import concourse.mybir as mybir
import concourse.tile as tile
from concourse.vector_clock import ScopedClock

# ---------------------------------------------------------------------------
# Workaround: this walrus build accepts only ONE semaphore wait on an SP
# Drain, but TileContext's tail drain carries one wait per live semaphore.
# Split them across a chain of drains.
# ---------------------------------------------------------------------------


def _patched_drain_and_barrier(self, tick_clock, wait_clock):
    nc = self.nc
    drain_inst = nc.sync.drain()
    wait_clock.add_sem_waits(
        drain_inst.ins, ScopedClock({None: tick_clock.global_clock})
    )
    waits = list(drain_inst.ins.sync_info.on_wait)
    if len(waits) > 1:
        drain_inst.ins.sync_info = bass_rust.SyncInfo(
            on_wait=waits[:1], on_update=list(drain_inst.ins.sync_info.on_update)
        )
        for i in range(1, len(waits)):
            extra = nc.sync.drain()
            extra.ins.sync_info = bass_rust.SyncInfo(
                on_wait=waits[i : i + 1], on_update=[]
            )
    nc.all_engine_barrier()
    assert self.sems is not None
    popped = nc._tile_sem_poison_stack.pop()
    assert popped is self._sem_poison
    nc.clear_and_free_semaphores(list(self.sems.allocated().values()))
    nc.all_engine_barrier()


tile.TileContext._drain_and_barrier = _patched_drain_and_barrier


def _split_multi_waits(nc):
    """This walrus build accepts at most one semaphore wait per instruction.
    Hoist extra waits onto standalone EventSemaphore instructions inserted
    just before the owning instruction in the same engine's stream."""
    uid = [0]
    for f in nc.m.functions:
        for bb in f.blocks:
            out = []
            for inst in bb.instructions:
                si = inst.sync_info
                if si is not None and len(si.on_wait) > 1:
                    waits = list(si.on_wait)
                    for w in waits[:-1]:
                        nop = mybir.InstEventSemaphore(
                            name=f"I-waitsplit-{uid[0]}", ins=[], outs=[]
                        )
                        uid[0] += 1
                        nop.engine = inst.engine
                        nop.sync_info = bass_rust.SyncInfo(
                            on_wait=[w], on_update=[]
                        )
                        out.append(nop)
                    inst.sync_info = bass_rust.SyncInfo(
                        on_wait=[waits[-1]], on_update=list(si.on_update)
                    )
                out.append(inst)
            bb.instructions[:] = out

# ---------------------------------------------------------------------------

B, R, H, IN = 64, 32, 1024, 2048
NCORES = 8
BL = B // NCORES  # examples per core
BR = BL * R  # 256 rows per core
KC = IN // 128  # 16 contraction chunks
MC = H // 128  # 8 h chunks
NEG = -1.0e30
WSCALE = 64.0  # fp8 weight pre-scale (power of two)

F32 = mybir.dt.float32
BF16 = mybir.dt.bfloat16


def build_program(mode="fp8", zero_bias=True):
    """Build the per-core Bass program. mode selects the dtype of the
    big-GEMM operands (weights + transposed activations):
    fp8 (DoubleRow, weights pre-scaled), bf16, or f32r."""
    if mode == "fp8":
        xdt = mybir.dt.float8e4
        step = 2
        pmode = mybir.MatmulPerfMode.DoubleRow
        sinv = 1.0 / WSCALE
    else:
        xdt = mybir.dt.float32r if mode == "f32r" else BF16
        step = 1
        pmode = None
        sinv = 1.0

    nc = bass.Bass()
    qt_d = nc.dram_tensor("qt", [128, KC, BR], xdt, kind="ExternalInput")
    ht_d = nc.dram_tensor("ht", [128, KC, BR], xdt, kind="ExternalInput")
    hn_d = nc.dram_tensor("hn", [128, 2, IN], BF16, kind="ExternalInput")
    wh_d = nc.dram_tensor("wh", [MC, 128, 2, KC, 128], xdt, kind="ExternalInput")
    wq_d = nc.dram_tensor("wq", [MC, 128, 2, KC, 128], xdt, kind="ExternalInput")
    b_d = {
        n: nc.dram_tensor(n, [128, MC], F32, kind="ExternalInput")
        for n in ("bhy", "bhg", "bqy", "bqg")
    }
    watt_d = nc.dram_tensor("watt", [128, MC], F32, kind="ExternalInput")
    mask_d = nc.dram_tensor("mask", [128, 128], F32, kind="ExternalInput")
    ident_d = nc.dram_tensor("ident", [128, 128], F32, kind="ExternalInput")
    feat_d = nc.dram_tensor("feat", [2, 128, IN], BF16, kind="ExternalOutput")

    ACT = mybir.ActivationFunctionType

    with tile.TileContext(nc) as tc:
        with (
            tc.tile_pool(name="big", bufs=1) as big,
            tc.tile_pool(name="wts", bufs=6) as wts,
            tc.tile_pool(name="tmp", bufs=3) as tmp,
            tc.tile_pool(name="sm", bufs=1) as sm,
        ):
            # ques-transposed activations: needed first; 2 chunks so the
            # first matmuls can start after half the transfer
            qt = big.tile([128, KC, BR], xdt, tag="qt")
            for h2 in range(2):
                ks = slice(8 * h2, 8 * (h2 + 1))
                nc.sync.dma_start(qt[:, ks, :], qt_d[:, ks, :])

            EDT = BF16
            he = big.tile([128, MC, BR], EDT, tag="he")
            he2 = big.tile([128, MC, BR], EDT, tag="he2")
            qew = big.tile([128, MC, BR], EDT, tag="qew")
            qe2 = big.tile([128, MC, BR], EDT, tag="qe2")

            # consts on the scalar HWDGE ring (keeps the sync ring free
            # for the weight stream)
            bsb = {}
            for n in ("bqy", "bqg", "bhy", "bhg"):
                bsb[n] = sm.tile([128, MC], F32, name=n, tag=n)
                nc.scalar.dma_start(bsb[n][:], b_d[n][:])
            watt = sm.tile([128, MC], F32, tag="watt")
            nc.scalar.dma_start(watt[:], watt_d[:])
            mask = sm.tile([128, 128], F32, tag="mask")
            nc.scalar.dma_start(mask[:], mask_d[:])
            ident = sm.tile([128, 128], F32, tag="ident")
            nc.scalar.dma_start(ident[:], ident_d[:])

            with (
                tc.tile_pool(name="pse", bufs=2, space="PSUM") as pse,
                tc.tile_pool(name="psnd", bufs=1, space="PSUM") as psnd,
            ):
                num_ps = [psnd.tile([128, 128], F32, name=f"num{g}", tag=f"num{g}") for g in range(2)]
                den_ps = [psnd.tile([128, 128], F32, name=f"den{g}", tag=f"den{g}") for g in range(2)]

                def gated(xt, w_dram, by, bg, m):
                    """One contiguous y+g weight DMA; big GEMM pair.
                    Returns (ty, tg) [128, BR]: ty = tanh branch, tg = the
                    (64x-scaled in fp8 mode) leaky_relu branch."""
                    wt = wts.tile([128, 2, KC, 128], xdt, tag="wt")
                    nc.sync.dma_start(wt[:], w_dram[m])
                    psy = pse.tile([128, BR], F32, tag="psy")
                    for k in range(0, KC, step):
                        nc.tensor.matmul(
                            psy[:],
                            wt[:, 0, k : k + step, :] if step == 2 else wt[:, 0, k, :],
                            xt[:, k : k + step, :] if step == 2 else xt[:, k, :],
                            start=(k == 0), stop=(k + step == KC),
                            perf_mode=pmode,
                        )
                    psg = pse.tile([128, BR], F32, tag="psg")
                    for k in range(0, KC, step):
                        nc.tensor.matmul(
                            psg[:],
                            wt[:, 1, k : k + step, :] if step == 2 else wt[:, 1, k, :],
                            xt[:, k : k + step, :] if step == 2 else xt[:, k, :],
                            start=(k == 0), stop=(k + step == KC),
                            perf_mode=pmode,
                        )
                    ty = tmp.tile([128, BR], F32, tag="ty")
                    nc.scalar.activation(
                        ty[:], psy[:], ACT.Tanh, bias=by[:, m : m + 1], scale=sinv
                    )
                    t1 = tmp.tile([128, BR], F32, tag="t1")
                    tg = tmp.tile([128, BR], F32, tag="tg")
                    if zero_bias:
                        # leaky_relu(sx) = s*leaky_relu(x): keep the scale
                        nc.vector.tensor_scalar_mul(t1[:], psg[:], 0.01)
                        nc.vector.tensor_max(tg[:], psg[:], t1[:])
                    else:
                        # non-scaled path only (bf16/f32r modes)
                        nc.vector.tensor_scalar(
                            t1[:], psg[:], bg[:, m : m + 1], 0.01,
                            op0=mybir.AluOpType.add, op1=mybir.AluOpType.mult,
                        )
                        nc.vector.tensor_scalar_add(tg[:], psg[:], bg[:, m : m + 1])
                        nc.vector.tensor_max(tg[:], tg[:], t1[:])
                    return ty, tg

                # ques embeddings (first: only needs qt + wq)
                for m in range(MC):
                    ty, tg = gated(qt, wq_d, bsb["bqy"], bsb["bqg"], m)
                    # qew = ty * (watt/WSCALE^2) * tg_scaled  -> qew_true/64
                    nc.vector.scalar_tensor_tensor(
                        qew[:, m, :], ty[:], watt[:, m : m + 1], tg[:],
                        op0=mybir.AluOpType.mult, op1=mybir.AluOpType.mult,
                    )
                    qe = tmp.tile([128, BR], F32, tag="qe")
                    nc.vector.tensor_mul(qe[:], ty[:], tg[:])
                    # (qe_scaled/64)^2 = qe_true^2
                    nc.scalar.activation(qe2[:, m, :], qe[:], ACT.Square, scale=sinv)
                    if m == 0:
                        # hist-transposed activations: stream during ques phase
                        ht = big.tile([128, KC, BR], xdt, tag="ht")
                        nc.sync.dma_start(ht[:], ht_d[:])
                        # feat inputs on the scalar ring
                        hn = big.tile([128, 2, IN], BF16, tag="hn")
                        nc.scalar.dma_start(hn[:], hn_d[:])

                # hist embeddings + num/den accumulation per chunk
                for m in range(MC):
                    ty, tg = gated(ht, wh_d, bsb["bhy"], bsb["bhg"], m)
                    nc.vector.tensor_mul(he[:, m, :], ty[:], tg[:])
                    nc.scalar.activation(he2[:, m, :], he[:, m, :], ACT.Square, scale=sinv)
                    for g in range(2):
                        sl = slice(128 * g, 128 * (g + 1))
                        nc.tensor.matmul(
                            num_ps[g][:], qew[:, m, sl], he[:, m, sl],
                            start=(m == 0), stop=(m == MC - 1),
                        )
                        nc.tensor.matmul(
                            den_ps[g][:], qe2[:, m, sl], he2[:, m, sl],
                            start=(m == 0), stop=(m == MC - 1),
                        )

                # scores while num/den PSUM is still available
                sc = []
                for g in range(2):
                    sd = tmp.tile([128, 128], F32, tag="sd")
                    nc.scalar.activation(sd[:], den_ps[g][:], ACT.Sqrt)
                    rd = tmp.tile([128, 128], F32, tag="rd")
                    nc.vector.reciprocal(rd[:], sd[:])
                    s = sm.tile([128, 128], F32, name=f"sc{g}", tag=f"sc{g}")
                    nc.vector.tensor_mul(s[:], num_ps[g][:], rd[:])
                    nc.vector.tensor_add(s[:], s[:], mask[:])
                    sc.append(s)

            # attention tail + feat
            with (
                tc.tile_pool(name="psa", bufs=1, space="PSUM") as psa,
                tc.tile_pool(name="psf", bufs=4, space="PSUM") as psf,
            ):
                for g in range(2):
                    s = sc[g]
                    att = sm.tile([128, 128], F32, name=f"att{g}", tag=f"att{g}")
                    nc.vector.memset(att[:], 0.0)
                    rs = sm.tile([128, 1], F32, name=f"rs{g}", tag=f"rs{g}")
                    for e in range(4):
                        bl = slice(32 * e, 32 * (e + 1))
                        nc.scalar.activation(att[bl, bl], s[bl, bl], ACT.Exp)
                        nc.vector.reduce_sum(
                            rs[bl, :], att[bl, bl], axis=mybir.AxisListType.X
                        )
                    rrs = sm.tile([128, 1], F32, name=f"rrs{g}", tag=f"rrs{g}")
                    nc.vector.reciprocal(rrs[:], rs[:])
                    nc.vector.tensor_scalar_mul(att[:], att[:], rrs[:])
                    atp = psa.tile([128, 128], F32, tag="atp")
                    nc.tensor.transpose(atp[:], att[:], ident[:])
                    atb = sm.tile([128, 128], BF16, name=f"atb{g}", tag=f"atb{g}")
                    nc.scalar.copy(atb[:], atp[:])
                    for c2 in range(2):
                        fsb = tmp.tile([128, 1024], BF16, tag="fsb")
                        for half in range(2):
                            c = 2 * c2 + half
                            cs = slice(512 * c, 512 * (c + 1))
                            fps = psf.tile([128, 512], F32, tag="fps")
                            nc.tensor.matmul(
                                fps[:], atb[:], hn[:, g, cs], start=True, stop=True
                            )
                            dst = fsb[:, 512 * half : 512 * (half + 1)]
                            if half == 0:
                                nc.scalar.copy(dst, fps[:])
                            else:
                                nc.vector.tensor_copy(dst, fps[:])
                        nc.sync.dma_start(
                            feat_d[g, :, 1024 * c2 : 1024 * (c2 + 1)], fsb[:]
                        )

    _split_multi_waits(nc)
    return nc


# ---------------------------------------------------------------------------
# Host side
# ---------------------------------------------------------------------------

_PROG_CACHE = {}


def _get_prog(mode, zero_bias):
    key = (mode, zero_bias)
    if key not in _PROG_CACHE:
        _PROG_CACHE[key] = build_program(mode, zero_bias)
    return _PROG_CACHE[key]


def _xnp(mode):
    if mode == "fp8":
        return ml_dtypes.float8_e4m3
    return np.float32 if mode == "f32r" else ml_dtypes.bfloat16


def _prep_shared(W_hy, b_hy, W_hg, b_hg, W_qy, b_qy, W_qg, b_qg, W_att, mode):
    xnp = _xnp(mode)
    ws = WSCALE if mode == "fp8" else 1.0

    def reblock(W):
        # [IN, H] -> [128, MC, KC, 128]; Wr[p, m, k, h] = W[128k+p, 128m+h]
        return (W.reshape(KC, 128, MC, 128) * ws).transpose(1, 2, 0, 3).astype(xnp)

    def bvec(b):
        return np.ascontiguousarray(b.reshape(MC, 128).T).astype(np.float32)

    m32 = np.where(
        np.arange(32)[None, :] <= np.arange(32)[:, None], 0.0, NEG
    ).astype(np.float32)
    mask = np.tile(m32, (4, 4))
    # [MC, 128, 2, KC, 128]
    wh = np.ascontiguousarray(
        np.stack([reblock(W_hy), reblock(W_hg)], axis=2).transpose(1, 0, 2, 3, 4)
    )
    wq = np.ascontiguousarray(
        np.stack([reblock(W_qy), reblock(W_qg)], axis=2).transpose(1, 0, 2, 3, 4)
    )
    watt = bvec(W_att)
    if mode == "fp8":
        watt = watt / (ws * ws)
    shared = {
        "wh": wh,
        "wq": wq,
        "bhy": bvec(b_hy),
        "bhg": bvec(b_hg),
        "bqy": bvec(b_qy),
        "bqg": bvec(b_qg),
        "watt": watt,
        "mask": np.ascontiguousarray(mask),
        "ident": np.eye(128, dtype=np.float32),
    }
    return shared, xnp


def kernel(
    hist, ques, W_hy, b_hy, W_hg, b_hg, W_qy, b_qy, W_qg, b_qg, W_att, b_att,
    mode="fp8", trace=False,
):
    from concourse.bass_utils import run_bass_kernel_spmd

    hist = np.asarray(hist, np.float32)
    ques = np.asarray(ques, np.float32)
    zero_bias = all(
        not np.any(np.asarray(b)) for b in (b_hy, b_hg, b_qy, b_qg)
    )
    if mode == "fp8" and not zero_bias:
        mode = "bf16"  # scaled-lrelu trick needs zero gate bias
    nc = _get_prog(mode, zero_bias)
    shared, xnp = _prep_shared(
        np.asarray(W_hy, np.float32), np.asarray(b_hy, np.float32),
        np.asarray(W_hg, np.float32), np.asarray(b_hg, np.float32),
        np.asarray(W_qy, np.float32), np.asarray(b_qy, np.float32),
        np.asarray(W_qg, np.float32), np.asarray(b_qg, np.float32),
        np.asarray(W_att, np.float32), mode,
    )
    in_maps = []
    for c in range(NCORES):
        hs = hist[c * BL : (c + 1) * BL].reshape(BR, IN)
        qs = ques[c * BL : (c + 1) * BL].reshape(BR, IN)
        im = dict(shared)
        # [128, KC, BR]; qt[p, k, b] = qs[b, 128k+p]
        im["qt"] = np.ascontiguousarray(
            qs.T.reshape(KC, 128, BR).transpose(1, 0, 2)
        ).astype(xnp)
        im["ht"] = np.ascontiguousarray(
            hs.T.reshape(KC, 128, BR).transpose(1, 0, 2)
        ).astype(xnp)
        # [128, 2, IN]; hn[p, t, d] = hs[128t+p, d]
        im["hn"] = np.ascontiguousarray(
            hs.reshape(2, 128, IN).transpose(1, 0, 2)
        ).astype(ml_dtypes.bfloat16)
        in_maps.append(im)

    res = run_bass_kernel_spmd(
        nc, in_maps, core_ids=list(range(NCORES)), trace=trace
    )
    feat = np.concatenate(
        [
            r["feat"].astype(np.float32).reshape(BL, R, IN)
            for r in res.results
        ],
        axis=0,
    )
    if trace:
        return feat, res
    return feat
